# revision 10
# baseline (speedup 1.0000x reference)
"""7x7 'same' 2D convolution over [128, 512, 512] f32, data-parallel on 8 NeuronCores.

Banded-Toeplitz formulation on the TensorEngine with 32x32 array
packing: the PE array runs as 16 independent 32x32 tiles
(tile_position=(32r, 32q)), each computing a 26-row output block
    out[o0+m, j] = sum_v sum_p T_v[p, m] * xpad[o0+p, j+v]
with T_v[p, m] = w[p-m, v] (band, p<32, m<26) -- 2x the useful-MAC
density of 64x64 packing (7/32 vs 7/64). The 7 column taps (v)
accumulate into PSUM. A main "job" covers one image's rows 0..415:
tile (r, q) reads K partitions [32r, 32r+32) of slab q and writes PSUM
bank r, partitions [32q, 32q+26) = out rows 104q + 26r + [0,26).
Rows 416..511 of four consecutive images batch into one tail job
(tile (r, c) = image c's slab 4, bank r, partitions [32c, 32c+26)).

Inputs are cast to fp16 and pre-staged host-side into the SBUF slab
layout (partition 32r+p, slab q, col) = padded row 104q + 26r + p
(6-row halos duplicated), so each image loads with one contiguous
663KB DMA; all 16 loads are prefetched up front. Accumulation is
fp32; PSUM banks are evacuated f32->bf16 into a [128, 4*512] SBUF
tile (col block r <- bank r) with the copy engine rotating
vector/scalar/gpsimd, then stored compactly: per job, 4 DMAs (one per
26-partition group q) write only the valid partitions as 26 runs of
4KB, rotating the scalar/gpsimd/vector/sync rings. Output HBM layout
og[img, q, p, r, c] / ot[t, c, p, r, col] is un-permuted on the host.
"""

import numpy as np

B, H, W = 128, 512, 512
KS = 7
PAD = (KS - 1) // 2          # 3
HP = H + 2 * PAD             # 518
PADROWS = 526                # 518 padded rows + 8 zero rows (layout slack)
N_CORES = 8
PER_CORE = B // N_CORES      # 16
TS = 26                      # output rows per 32x32 tile (32 - 6)
NS = 5                       # slabs per image (4 main + 1 tail)


def _build_program():
    import concourse.bass as bass
    import concourse.tile as tile
    from concourse import bacc, mybir

    f16 = mybir.dt.float16
    bf16 = mybir.dt.bfloat16
    f32 = mybir.dt.float32

    nc = bacc.Bacc("TRN2", target_bir_lowering=False, debug=False,
                   num_devices=N_CORES)
    x_ext = nc.declare_dram_parameter("x", [PER_CORE, 128, NS * HP], f16,
                                      isOutput=False)
    t_ext = nc.declare_dram_parameter("toep", [128, KS * TS], f16,
                                      isOutput=False)
    # og[img, q, p, r, c]: out row 104q + 26r + p (p < 26), col c
    og_ext = nc.declare_dram_parameter("og", [PER_CORE, 4, TS, 4, W],
                                       bf16, isOutput=True)
    # ot[t, c, p, r, col]: img 4t + c, out row 416 + 26r + p (clip < 512)
    ot_ext = nc.declare_dram_parameter("ot", [PER_CORE // 4, 4, TS, 4, W],
                                       bf16, isOutput=True)

    with tile.TileContext(nc) as tc:
        with (
            tc.tile_pool(name="toep", bufs=1) as toep_pool,
            tc.tile_pool(name="xs", bufs=PER_CORE) as xs_pool,
            tc.tile_pool(name="psum", bufs=8, space="PSUM") as psum_pool,
            tc.tile_pool(name="outs", bufs=6) as out_pool,
        ):
            toep_sb = toep_pool.tile([128, KS * TS], f16)
            nc.sync.dma_start(out=toep_sb[:], in_=t_ext[:])

            # prefetch every image up front; the sync ring drains in order
            stages = []
            for img in range(PER_CORE):
                st = xs_pool.tile([128, NS * HP], f16, name=f"st{img}",
                                  tag="st")
                nc.sync.dma_start(out=st[:], in_=x_ext[img])
                stages.append(st)

            copy_engines = None
            store_rings = None

            def run_job(movers, dst):
                """movers[r][q] -> (stage, col_off); dst = og/ot slice fn."""
                nonlocal copy_engines, store_rings
                ps = [psum_pool.tile([128, W], f32, name=f"ps{r}", tag="acc")
                      for r in range(4)]
                for v in range(KS):
                    for r in range(4):
                        lhsT = toep_sb[32 * r:32 * r + 32, TS * v:TS * (v + 1)]
                        for q in range(4):
                            st, off = movers[r][q]
                            nc.tensor.matmul(
                                ps[r][32 * q:32 * q + TS, :],
                                lhsT,
                                st[32 * r:32 * r + 32, off + v:off + v + W],
                                start=(v == 0),
                                stop=(v == KS - 1),
                                tile_position=(32 * r, 32 * q),
                            )
                o_sb = out_pool.tile([128, 4 * W], bf16, name="o", tag="osb")
                for r in range(4):
                    eng = next(copy_engines)
                    if eng is nc.scalar:
                        eng.copy(o_sb[:, r * W:(r + 1) * W], ps[r][:])
                    else:
                        eng.tensor_copy(o_sb[:, r * W:(r + 1) * W], ps[r][:])
                # compact store: only the 26 valid rows of each 32-row
                # partition group, one DMA per group (26 x 4KB runs)
                for q in range(4):
                    ring = next(store_rings)
                    ring.dma_start(out=dst[q],
                                   in_=o_sb[32 * q:32 * q + TS, :])

            def cyc(seq):
                while True:
                    for e in seq:
                        yield e

            copy_engines = cyc([nc.vector, nc.scalar])
            store_rings = cyc([nc.gpsimd, nc.sync, nc.gpsimd, nc.scalar])

            for img in range(PER_CORE):
                movers = [[(stages[img], q * HP) for q in range(4)]
                          for _ in range(4)]
                run_job(movers, og_ext[img])
                if img % 4 == 3:
                    t = img // 4
                    movers = [[(stages[4 * t + c], 4 * HP) for c in range(4)]
                              for _ in range(4)]
                    run_job(movers, ot_ext[t])
    nc.finalize()
    return nc


def _host_prep(x, w):
    x = np.asarray(x, dtype=np.float32)
    w = np.asarray(w, dtype=np.float32)
    xpad = np.zeros((B, PADROWS, HP), dtype=np.float16)
    xpad[:, PAD:PAD + H, PAD:PAD + W] = x
    # slab layout: (partition 32r + p, slab q) -> padded row 104q + 26r + p
    P = np.arange(128)
    q = np.arange(NS)
    ridx = 104 * q[None, :] + 26 * (P[:, None] // 32) + (P[:, None] % 32)
    xslab = np.ascontiguousarray(
        xpad[:, ridx, :].reshape(B, 128, NS * HP))
    # Toeplitz band [32, 26] per tap, replicated on all four 32-row strips
    toep = np.zeros((128, KS * TS), dtype=np.float16)
    w16 = w.astype(np.float16)
    m = np.arange(TS)
    for s in range(4):
        for v in range(KS):
            for d in range(KS):
                toep[32 * s + m + d, TS * v + m] = w16[d, v]
    return xslab, toep


def _execute(x, w, **run_kwargs):
    from concourse.bass_utils import run_bass_kernel_spmd

    xslab, toep = _host_prep(x, w)
    nc = _build_program()
    in_maps = [
        {"x": xslab[c * PER_CORE:(c + 1) * PER_CORE], "toep": toep}
        for c in range(N_CORES)
    ]
    last_err = None
    for _attempt in range(3):
        try:
            res = run_bass_kernel_spmd(nc, in_maps,
                                       core_ids=list(range(N_CORES)),
                                       **run_kwargs)
            break
        except Exception as e:  # transient NRT execute flakes -> retry
            last_err = e
    else:
        raise last_err
    out = np.empty((B, H, W), dtype=np.float32)
    for c in range(N_CORES):
        sl = slice(c * PER_CORE, (c + 1) * PER_CORE)
        og = np.asarray(res.results[c]["og"], dtype=np.float32)
        ot = np.asarray(res.results[c]["ot"], dtype=np.float32)
        # og [img, q, p, r, c] -> row 104q + 26r + p
        out[sl, :416, :] = og.transpose(0, 1, 3, 2, 4).reshape(
            PER_CORE, 416, W)
        # ot [t, c, p, r, col] -> img 4t + c, row 416 + 26r + p (clip 512)
        tail = ot.transpose(0, 1, 3, 2, 4).reshape(PER_CORE // 4, 4, 104, W)
        out[sl, 416:, :] = tail.reshape(PER_CORE, 104, W)[:, :96, :]
    return out, res


def kernel(x, w):
    out, _ = _execute(x, w)
    return out


# revision 11
# speedup vs baseline: 1.2006x; 1.2006x over previous
"""7x7 'same' 2D convolution over [128, 512, 512] f32, data-parallel on 8 NeuronCores.

Banded-Toeplitz formulation on the TensorEngine with 64x64 array
packing: the PE array runs as 4 independent 64x64 tiles
(tile_position=(64r, 64g)), each computing a 58-row output block
    out[i0+m, j] = sum_v sum_{r'} T_v[r', m] * xpad[i0+r', j+v]
with T_v[r', m] = w[r'-m, v] (band, r'<64, m<58). The 7 column taps (v)
accumulate into PSUM; tile (s, r2, g2) covers out rows
232s + 116g2 + 58r2 + [0,58) and writes PSUM bank (s, r2), partitions
[64g2, 64g2+58). One 4-tile tap sweep streams in one N=512 matmul
time (the TensorE issues an LDWEIGHTS+MATMUL pair every ~34ns, so
4 pairs per 237ns sweep leaves issue headroom; finer 32x32 packing
is issue-bound and slower). 8 tiles cover rows 0..463 of an image;
rows 464..511 of four consecutive images batch into one 4-tile
"tail" group.

Inputs are cast to fp16 and pre-staged host-side into the SBUF slab
layout (partition 64r+p, slab q, col) = padded row 116q + 58r + p
(slab 4 = tail rows, duplicated on both strips). All images' loads
are prefetched up front on the sync ring (two DMAs per image so the
s=0 matmuls depend only on the first). Accumulation is fp32; outputs
are stored as raw bf16 PSUM-bank dumps (vector/scalar copies, stores
rotating the gpsimd/sync/scalar rings) and un-permuted on the host.
"""

import numpy as np

B, H, W = 128, 512, 512
KS = 7
PAD = (KS - 1) // 2          # 3
HP = H + 2 * PAD             # 518
N_CORES = 8
PER_CORE = B // N_CORES      # 16
TS = 58                      # output rows per 64x64 tile (64 - 6)
NS = 5                       # slabs per image (4 group-1 + 1 tail)
TAILM = H - 8 * TS           # 48 tail output rows per image
TAILK = TAILM + KS - 1       # 54


def _build_program():
    import concourse.bass as bass
    import concourse.tile as tile
    from concourse import bacc, mybir

    f16 = mybir.dt.float16
    bf16 = mybir.dt.bfloat16
    f32 = mybir.dt.float32

    nc = bacc.Bacc("TRN2", target_bir_lowering=False, debug=False,
                   num_devices=N_CORES)
    x_ext = nc.declare_dram_parameter("x", [PER_CORE, 128, NS * HP], f16,
                                      isOutput=False)
    t_ext = nc.declare_dram_parameter("toep", [128, KS * TS], f16,
                                      isOutput=False)
    # og[img, s, r] = dump of PSUM bank (s, r):
    #   row 64g+p  ->  out row 232s + 116g + 58r + p   (valid p < 58)
    og_ext = nc.declare_dram_parameter("og", [PER_CORE, 2, 2, 128, W],
                                       bf16, isOutput=True)
    # ot[tg, r] = tail bank dump: row 64g+p -> img 4tg + 2g + r,
    #   out row 464 + p  (valid p < 48)
    ot_ext = nc.declare_dram_parameter("ot", [PER_CORE // 4, 2, 128, W],
                                       bf16, isOutput=True)

    with tile.TileContext(nc) as tc:
        with (
            tc.tile_pool(name="toep", bufs=1) as toep_pool,
            tc.tile_pool(name="xa", bufs=PER_CORE) as xa_pool,
            tc.tile_pool(name="xb", bufs=PER_CORE) as xb_pool,
            tc.tile_pool(name="psum", bufs=8, space="PSUM") as psum_pool,
            tc.tile_pool(name="outs", bufs=12) as out_pool,
        ):
            toep_sb = toep_pool.tile([128, KS * TS], f16)
            nc.sync.dma_start(out=toep_sb[:], in_=t_ext[:])

            # prefetch every image's two slab groups up front
            sta, stb = [], []
            for img in range(PER_CORE):
                st_a = xa_pool.tile([128, 2 * HP], f16, name=f"sta{img}",
                                    tag="sta")
                nc.sync.dma_start(out=st_a[:], in_=x_ext[img, :, :2 * HP])
                st_b = xb_pool.tile([128, 3 * HP], f16, name=f"stb{img}",
                                    tag="stb")
                nc.sync.dma_start(out=st_b[:], in_=x_ext[img, :, 2 * HP:])
                sta.append(st_a)
                stb.append(st_b)

            def cyc(seq):
                while True:
                    for e in seq:
                        yield e

            copy_engines = cyc([nc.vector, nc.scalar])
            store_rings = cyc([nc.gpsimd, nc.sync, nc.gpsimd, nc.scalar])

            def evac(ps, dst):
                o_sb = out_pool.tile([128, W], bf16, name="o", tag="osb")
                eng = next(copy_engines)
                if eng is nc.scalar:
                    eng.copy(o_sb[:], ps[:])
                else:
                    eng.tensor_copy(o_sb[:], ps[:])
                next(store_rings).dma_start(out=dst, in_=o_sb[:])

            for img in range(PER_CORE):
                # s-groups sequential: only 2 PSUM banks live per group,
                # so allocation never stalls on evacuation of 4 banks.
                for s in range(2):
                    stage = sta[img] if s == 0 else stb[img]
                    ps = [psum_pool.tile([128, W], f32, name=f"ps{r}",
                                         tag="acc") for r in range(2)]
                    for v in range(KS):
                        for g in range(2):
                            for r in range(2):
                                nc.tensor.matmul(
                                    ps[r][64 * g:64 * g + TS, :],
                                    toep_sb[64 * r:64 * r + 64,
                                            TS * v:TS * (v + 1)],
                                    stage[64 * r:64 * r + 64,
                                          g * HP + v:g * HP + v + W],
                                    start=(v == 0),
                                    stop=(v == KS - 1),
                                    tile_position=(64 * r, 64 * g),
                                )
                    for r in range(2):
                        evac(ps[r], og_ext[img, s, r])

                    if s == 0 and img % 4 == 3:
                        # tail group between the two s-groups so its
                        # evacuations interleave with s=1 compute
                        tg = img // 4
                        pst = [psum_pool.tile([128, W], f32,
                                              name=f"pt{r}", tag="acc")
                               for r in range(2)]
                        for v in range(KS):
                            for j in range(4):
                                r, g = j % 2, j // 2
                                nc.tensor.matmul(
                                    pst[r][64 * g:64 * g + TAILM, :],
                                    toep_sb[64 * r:64 * r + TAILK,
                                            TS * v:TS * v + TAILM],
                                    stb[4 * tg + j][
                                        64 * r:64 * r + TAILK,
                                        2 * HP + v:2 * HP + v + W],
                                    start=(v == 0),
                                    stop=(v == KS - 1),
                                    tile_position=(64 * r, 64 * g),
                                )
                        for r in range(2):
                            evac(pst[r], ot_ext[tg, r])
    nc.finalize()
    return nc


def _host_prep(x, w):
    x = np.asarray(x, dtype=np.float32)
    w = np.asarray(w, dtype=np.float32)
    # padded images with extra zero rows (slab-4 strip-1 reads to 585)
    xpad = np.zeros((B, 586, HP), dtype=np.float16)
    xpad[:, PAD:PAD + H, PAD:PAD + W] = x
    # slab layout: (p, q) -> padded row 116q + 58*(p//64) + p%64;
    # slab 4 = tail rows 464+, duplicated on both 64-row strips
    p = np.arange(128)
    q = np.arange(NS)
    ridx = 116 * q[None, :] + 58 * (p[:, None] // 64) + (p[:, None] % 64)
    ridx[:, 4] = 464 + (p % 64)
    xslab = np.ascontiguousarray(
        xpad[:, ridx, :].reshape(B, 128, NS * HP))
    # Toeplitz band [64, 58] per tap, replicated on both partition strips
    toep = np.zeros((128, KS * TS), dtype=np.float16)
    w16 = w.astype(np.float16)
    idx = np.arange(TS)
    for st in range(2):
        for v in range(KS):
            for d in range(KS):
                toep[64 * st + idx + d, TS * v + idx] = w16[d, v]
    return xslab, toep


def _execute(x, w, **run_kwargs):
    from concourse.bass_utils import run_bass_kernel_spmd

    xslab, toep = _host_prep(x, w)
    nc = _build_program()
    in_maps = [
        {"x": xslab[c * PER_CORE:(c + 1) * PER_CORE], "toep": toep}
        for c in range(N_CORES)
    ]
    last_err = None
    for _attempt in range(3):
        try:
            res = run_bass_kernel_spmd(nc, in_maps,
                                       core_ids=list(range(N_CORES)),
                                       **run_kwargs)
            break
        except Exception as e:  # transient NRT execute flakes -> retry
            last_err = e
    else:
        raise last_err
    out = np.empty((B, H, W), dtype=np.float32)
    for c in range(N_CORES):
        sl = slice(c * PER_CORE, (c + 1) * PER_CORE)
        og = np.asarray(res.results[c]["og"], dtype=np.float32)
        ot = np.asarray(res.results[c]["ot"], dtype=np.float32)
        og6 = og.reshape(PER_CORE, 2, 2, 2, 64, W)[:, :, :, :, :TS, :]
        # [img, s, r, g, p, w] -> row = 232s + 116g + 58r + p
        out[sl, :8 * TS, :] = og6.transpose(0, 1, 3, 2, 4, 5).reshape(
            PER_CORE, 8 * TS, W)
        ot5 = ot.reshape(PER_CORE // 4, 2, 2, 64, W)[:, :, :, :TAILM, :]
        # [tg, r, g, p, w] -> img 4tg + 2g + r, row 464 + p
        out[sl, 8 * TS:, :] = ot5.transpose(0, 2, 1, 3, 4).reshape(
            PER_CORE, TAILM, W)
    return out, res


def kernel(x, w):
    out, _ = _execute(x, w)
    return out
